# revision 29
# baseline (speedup 1.0000x reference)
"""Trainium2 Bass kernel for nn_HeteroModel (2-layer hetero GraphSAGE).

Device strategy (per core, nodes dst-sharded 8 ways):
- Host does integer/index preprocessing and the edge-major feature gather
  (pure data layout); all float math runs on device.
- Segment-mean on device as PE matmuls: for each <=128-edge chunk,
  A^T[:, cols] = Z_chunk^T @ M_chunk with Z (gathered src features, fp8
  e4m3, scaled by 16/deg[dst] on the host during the gather) stationary
  and the binary one-hot M (fp8, exact) moving.  The stationary AP is a
  128-column window over a contiguous [128, nc_tot*96] fp8 stream (the
  32 trailing columns alias the next chunk), which keeps DMA contiguous
  and triggers fast-weight-load (NumWeights==128, non-fp32).  PSUM rows
  96..127 are garbage and never drained.
- Nodes are assigned to chunks per-core by best-fit-decreasing vector
  bin packing (each bin <=128 edges in EVERY relation), sharing one
  node-count-per-bin profile across cores/relations so a single program
  serves all 8 cores; the host permutes own features / unpermutes the
  output accordingly.  This packs chunks to ~124/128 edges vs ~105 for
  a consecutive-node schedule.
- The 16x scale folded into Z is undone by pre-scaling Wn by 1/16 on the
  host (exact exponent shift).
- Dense part per relation: out^T = tanh(Ws^T h^T + Wn^T A^T + b) via
  PSUM-accumulated bf16 matmuls over 512-column tiles (weights padded to
  128 columns for FWL), ScalarE tanh with per-partition bias, DVE
  relation-mean accumulation, outputs streamed per 512-column round.
  h^T arrives pre-transposed (own_t) and the result leaves transposed
  (out_t); the device does no transposes at all.
- One compiled program serves both layers (layer-2 weights zero-padded);
  host runs it twice, redistributing h1 between launches.
"""

import sys

if "/opt/trn_rl_repo" not in sys.path:
    sys.path.insert(0, "/opt/trn_rl_repo")

import numpy as np
import ml_dtypes

P = 128
R = 3
N_REAL = 50000
D = 96
DO = 64
NCORES = 8
ZBLK = 96            # chunks per z-feature DMA block (steady state)
ZBLK0 = 16           # first block kept small so the PE starts sooner
ZPAD = P - D         # stationary-window overhang past the last chunk
PSUM_COLS = 512      # psum bank columns per aggregation group
DENSE_G = 4          # dense tiles per psum round (N=512 = one PSUM bank)
ZSCALE = 16.0        # pow2 scale folded into z (and out of Wn)

F8 = ml_dtypes.float8_e4m3
BF16 = ml_dtypes.bfloat16

FULL_CFG = dict(n_real=N_REAL, npad=50176, ncores=NCORES)

_cache = {}
LAST_RESULTS = []


def _bfd_pack(degs, B, per, cap=128.0):
    """Best-fit-decreasing vector bin packing with swap repair.

    degs: [R, per] per-node degrees; B bins with a fixed node-count
    profile (per//B or per//B+1 slots).  Every bin must satisfy
    sum(deg_r) <= cap for each relation r."""
    base = per // B
    k = per - B * base
    slots = np.full(B, base, np.int64)
    slots[:k] += 1
    order = np.argsort(-degs.sum(axis=0), kind="stable")
    loads = np.zeros((B, degs.shape[0]))
    left = slots.copy()
    assign = np.empty(per, np.int64)
    for n in order:
        score = (loads + degs[:, n]).max(axis=1)
        score[left == 0] = np.inf
        b = int(np.argmin(score))
        assign[n] = b
        loads[b] += degs[:, n]
        left[b] -= 1

    for _ in range(300):
        viol = np.where(loads.max(axis=1) > cap)[0]
        if len(viol) == 0:
            break
        b = viol[np.argmax(loads[viol].max(axis=1))]
        members = np.where(assign == b)[0]
        done = False
        for a in members[np.argsort(-degs[:, members].sum(axis=0))]:
            da = degs[:, a]
            lb = loads[b] - da
            nodes = np.where(assign != b)[0]
            dc = degs[:, nodes]
            ok = (((lb[:, None] + dc).max(axis=0) <= cap)
                  & ((loads[assign[nodes]].T - dc + da[:, None]).max(axis=0)
                     <= cap))
            if ok.any():
                c = nodes[np.argmax(ok)]
                b2 = assign[c]
                loads[b] += degs[:, c] - da
                loads[b2] += da - degs[:, c]
                assign[a], assign[c] = b2, b
                done = True
                break
        if not done:
            return None, None, loads.max()
    if loads.max() > cap:
        return None, None, loads.max()
    return assign, slots, loads.max()


def _preprocess(src, dst, n_real, npad, ncores):
    per = npad // ncores
    r_ = src.shape[0]
    deg = np.zeros((r_, npad), np.int64)
    for r in range(r_):
        deg[r, :] += np.bincount(dst[r], minlength=npad)

    edges = [[None] * r_ for _ in range(ncores)]
    for c in range(ncores):
        lo, hi = c * per, (c + 1) * per
        for r in range(r_):
            m = (dst[r] >= lo) & (dst[r] < hi)
            es = src[r][m].astype(np.int64)
            ed = dst[r][m].astype(np.int64) - lo
            order = np.argsort(ed, kind="stable")
            edges[c][r] = (es[order], ed[order])

    # shared-bin packing: one node-count profile, per-core assignments
    worst_e = max(deg[:, c * per:(c + 1) * per].sum(axis=1).max()
                  for c in range(ncores))
    assigns = slots = None
    for target in (124.0, 123.0, 122.0, 120.0, 116.0, 110.0, 96.0, 64.0):
        B = max(1, int(np.ceil(worst_e / target)))
        if B > per:
            continue
        res = []
        ok = True
        for c in range(ncores):
            a, s, _ = _bfd_pack(deg[:, c * per:(c + 1) * per], B, per)
            if a is None:
                ok = False
                break
            res.append(a)
            slots = s
        if ok:
            assigns = res
            break
    assert assigns is not None, "bin packing failed"
    return edges, slots, assigns, deg


def _layout(edges, slots, assigns, deg, npad, ncores):
    per = npad // ncores
    r_ = len(edges[0])
    B = len(slots)
    q = np.concatenate([[0], np.cumsum(slots)])  # chunk -> column range

    groups = []  # (chunk_lo, chunk_hi, col_lo, col_hi), shared by relations
    lo = 0
    for i in range(B + 1):
        if i == B or q[i + 1] - q[lo] > PSUM_COLS:
            groups.append((lo, i, int(q[lo]), int(q[i])))
            lo = i
        if i == B:
            break

    placements = []
    m_alls = []
    perms = []
    for c in range(ncores):
        assign = assigns[c]
        # columns: nodes ordered by bin; perm[col] = local node id
        cols_order = np.argsort(assign, kind="stable")
        col_of = np.empty(per, np.int64)
        col_of[cols_order] = np.arange(per)
        perms.append(cols_order + c * per)

        g_idx, e_idx, s_idx, w_idx = [], [], [], []
        m_all = np.zeros((P, r_ * per), F8)
        counts = np.bincount(assign, minlength=B)
        assert (counts == slots).all()
        for r in range(r_):
            es, ed = edges[c][r]
            ecol = col_of[ed]
            eo = np.argsort(ecol, kind="stable")
            es, ed, ecol = es[eo], ed[eo], ecol[eo]
            ebin = assign[ed]
            ecnt = np.bincount(ebin, minlength=B)
            estart = np.concatenate([[0], np.cumsum(ecnt)])[:-1]
            slot = np.arange(len(es)) - estart[ebin]
            assert len(slot) == 0 or slot.max() < P
            g_idx.append(ebin + r * B)
            e_idx.append(slot)
            s_idx.append(es)
            w_idx.append((ZSCALE / np.maximum(
                deg[r, c * per + ed], 1)).astype(np.float32))
            m_all[slot, r * per + ecol] = F8(1.0)
        placements.append((np.concatenate(g_idx), np.concatenate(e_idx),
                           np.concatenate(s_idx), np.concatenate(w_idx)))
        m_alls.append(m_all)
    return B, groups, q, placements, m_alls, perms


def _gather_zfeat(h_full, placements, nc_tot):
    """Edge-major gather: z[slot, chunk, :] = h[src] * (16/deg[dst]), fp8."""
    out = []
    for g_idx, e_idx, s_idx, w_idx in placements:
        zf = np.zeros((P, nc_tot, D), np.float32)
        zf[e_idx, g_idx] = h_full[s_idx] * w_idx[:, None]
        z8 = np.zeros((P, nc_tot * D + ZPAD), F8)
        z8[:, :nc_tot * D] = zf.reshape(P, nc_tot * D).astype(F8)
        out.append(z8)
    return out


def _build(nblk, groups, q, per, ncores):
    from concourse import bacc, mybir, tile

    T = per // P
    nc_tot = R * nblk
    f32 = mybir.dt.float32
    bf16 = mybir.dt.bfloat16
    f8 = mybir.dt.float8e4
    AO = mybir.AluOpType
    AF = mybir.ActivationFunctionType

    nc = bacc.Bacc(
        "TRN2", target_bir_lowering=False, debug=False,
        enable_asserts=False, num_devices=ncores,
    )

    zfeat = nc.dram_tensor("zfeat", [P, nc_tot * D + ZPAD], f8,
                           kind="ExternalInput").ap()
    m_all = nc.dram_tensor("m_all", [P, R * per], f8, kind="ExternalInput").ap()
    own_t = nc.dram_tensor("own_t", [D, per], bf16, kind="ExternalInput").ap()
    # ws||wn packed into one tensor (one DMA): [:, :R*P]=ws, [:, R*P:]=wn
    wpk = nc.dram_tensor("wpk", [D, 2 * R * P], bf16,
                         kind="ExternalInput").ap()
    bbt = nc.dram_tensor("bbt", [D, R], f32, kind="ExternalInput").ap()
    out_t = nc.dram_tensor("out_t", [D, per], f32, kind="ExternalOutput").ap()

    with tile.TileContext(nc) as tc:
        with (
            tc.tile_pool(name="const", bufs=1) as cp,
            tc.tile_pool(name="zb", bufs=5) as zp,
            tc.tile_pool(name="tmp", bufs=3) as tmpp,
            tc.tile_pool(name="pa", bufs=4, space="PSUM") as pap,
            tc.tile_pool(name="po", bufs=2, space="PSUM") as pop,
        ):
            # SBUF-resident binary segment matrix.  Relation 0's slab goes
            # first on the scalar queue (critical path to the first matmul)
            # so the sync queue can start streaming zfeat immediately; the
            # rest of m and the dense-side constants follow on scalar.
            m_full = cp.tile([P, R * per], f8, tag="m_full", name="m_full")
            nc.scalar.dma_start(out=m_full[:, :per], in_=m_all[:, :per])

            wpkt = cp.tile([D, 2 * R * P], bf16, tag="wpk", name="wpk")
            nc.scalar.dma_start(out=wpkt[:], in_=wpk[:, :])
            wst = [wpkt[:, r * P:(r + 1) * P] for r in range(R)]
            wnt = [wpkt[:, (R + r) * P:(R + r + 1) * P] for r in range(R)]
            bbt_t = cp.tile([D, R], f32, tag="bbt", name="bbt")
            nc.scalar.dma_start(out=bbt_t[:], in_=bbt[:, :])
            bt = [bbt_t[:, r:r + 1] for r in range(R)]
            hT = cp.tile([D, per], bf16, tag="hT", name="hT")
            nc.scalar.dma_start(out=hT[:], in_=own_t[:, :])
            nc.scalar.dma_start(out=m_full[:, per:], in_=m_all[:, per:])

            AT = [cp.tile([D, per], bf16, tag=f"AT{r}", name=f"AT{r}")
                  for r in range(R)]
            accf = cp.tile([D, per], f32, tag="accf", name="accf")

            def emit_dense(r, t0):
                g_n = min(DENSE_G, T - t0)
                cl, ch = t0 * P, (t0 + g_n) * P
                w = ch - cl
                po = pop.tile([P, DENSE_G * P], f32, tag="po", name="po")
                nc.tensor.matmul(
                    out=po[:, :w], lhsT=wst[r],
                    rhs=hT[:, cl:ch], start=True, stop=False)
                nc.tensor.matmul(
                    out=po[:, :w], lhsT=wnt[r],
                    rhs=AT[r][:, cl:ch], start=False, stop=True)
                if r == 0:
                    nc.scalar.activation(
                        out=accf[:, cl:ch], in_=po[:D, :w],
                        func=AF.Tanh, bias=bt[r])
                else:
                    tm = tmpp.tile([D, DENSE_G * P], f32, tag="tm",
                                   name="tm")
                    nc.scalar.activation(
                        out=tm[:, :w], in_=po[:D, :w],
                        func=AF.Tanh, bias=bt[r])
                    nc.vector.tensor_add(
                        out=accf[:, cl:ch],
                        in0=accf[:, cl:ch], in1=tm[:, :w])
                if r == R - 1:
                    nc.vector.tensor_scalar(
                        out=accf[:, cl:ch], in0=accf[:, cl:ch],
                        scalar1=1.0 / R, scalar2=None, op0=AO.mult)
                    nc.scalar.dma_start(out=out_t[:, cl:ch],
                                        in_=accf[:, cl:ch])

            # dense rounds of relation r-1 are interleaved between relation
            # r's aggregation groups so the PE never idles on DMA/drain
            # boundaries; relation R-1's rounds trail at the end.
            zsbuf = ZBLK * D + ZPAD
            n_zdma = 0
            pending = []
            for r in range(R):
                cb = r * nblk
                zb = None
                zblk_lo = -1
                blk = 0
                for (g_lo, g_hi, col_lo, col_hi) in groups:
                    ncols = col_hi - col_lo
                    pa = pap.tile([P, PSUM_COLS], f32, tag="pa", name="pa")
                    for i in range(g_lo, g_hi):
                        gi = cb + i
                        if zb is None or gi >= zblk_lo + blk:
                            zblk_lo = gi
                            blk = ZBLK0 if n_zdma == 0 else ZBLK
                            blk = min(blk, nc_tot - zblk_lo)
                            zb = zp.tile([P, zsbuf], f8, tag="zb", name="zb")
                            nc.sync.dma_start(
                                out=zb[:, :blk * D + ZPAD],
                                in_=zfeat[:, zblk_lo * D:
                                          (zblk_lo + blk) * D + ZPAD])
                            n_zdma += 1
                        ns = int(q[i + 1] - q[i])
                        q0 = int(q[i]) - col_lo
                        cq = r * per + int(q[i])
                        off = (gi - zblk_lo) * D
                        nc.tensor.matmul(
                            out=pa[:, q0:q0 + ns],
                            lhsT=zb[:, off:off + P],
                            rhs=m_full[:, cq:cq + ns],
                            start=True, stop=True,
                        )
                    nc.vector.tensor_copy(
                        out=AT[r][:, col_lo:col_hi], in_=pa[:D, :ncols])
                    if pending:
                        emit_dense(*pending.pop(0))
                pending.extend((r, t0) for t0 in range(0, T, DENSE_G))
            for r, t0 in pending:
                emit_dense(r, t0)

    nc.compile()
    return nc


def kernel(x, src, dst, Ws1, Wn1, b1, Ws2, Wn2, b2, cfg=None):
    global LAST_RESULTS
    from concourse import bass_utils

    cfg = cfg or FULL_CFG
    n_real, npad, ncores = cfg["n_real"], cfg["npad"], cfg["ncores"]
    per = npad // ncores

    x = np.asarray(x, np.float32)
    src = np.asarray(src, np.int64)
    dst = np.asarray(dst, np.int64)

    edges, slots, assigns, deg = _preprocess(src, dst, n_real, npad, ncores)
    B, groups, q, placements, m_alls, perms = _layout(
        edges, slots, assigns, deg, npad, ncores)
    nc_tot = R * B

    key = (B, npad, ncores, len(groups), tuple(slots[:8]))
    if key not in _cache:
        _cache[key] = _build(B, groups, q, per, ncores)
    nc = _cache[key]

    def launch(h_full, h_t, Wsl, Wnl, bl):
        dpad = Wsl.shape[2]
        wpk = np.zeros((D, 2 * R * P), np.float32)
        bbt = np.zeros((D, R), np.float32)
        for r in range(R):
            wpk[:, r * P:r * P + dpad] = Wsl[r]
            wpk[:, (R + r) * P:(R + r) * P + dpad] = Wnl[r] / ZSCALE
            bbt[:dpad, r] = bl[r]
        wpk = wpk.astype(BF16)
        zf = _gather_zfeat(h_full, placements, nc_tot)
        in_maps = []
        for c in range(ncores):
            in_maps.append(dict(
                zfeat=zf[c], m_all=m_alls[c],
                own_t=np.ascontiguousarray(h_t[:, perms[c]]).astype(BF16),
                wpk=wpk, bbt=bbt,
            ))
        res = bass_utils.run_bass_kernel_spmd(nc, in_maps,
                                              core_ids=list(range(ncores)))
        LAST_RESULTS.append(res)
        full_t = np.empty((D, npad), np.float32)
        for c in range(ncores):
            full_t[:, perms[c]] = res.results[c]["out_t"]
        return full_t

    x_pad = np.zeros((npad, D), np.float32)
    x_pad[:n_real] = x
    x_t = np.ascontiguousarray(x_pad.T)

    LAST_RESULTS = []
    h1_t = launch(x_pad, x_t, np.asarray(Ws1, np.float32),
                  np.asarray(Wn1, np.float32), np.asarray(b1, np.float32))
    h1_t[:, n_real:] = 0.0
    h1 = np.ascontiguousarray(h1_t.T)
    out2_t = launch(h1, h1_t, np.asarray(Ws2, np.float32),
                    np.asarray(Wn2, np.float32), np.asarray(b2, np.float32))
    return np.ascontiguousarray(out2_t[:DO, :n_real].T)
